# revision 23
# baseline (speedup 1.0000x reference)
"""Trainium2 Bass kernel: uniform cubic B-spline (8 basis, order 3) + linear term.

The reference evaluates an 11-basis cubic B-spline on uniform knots
(spacing h=0.25 over [-1.75, 1.75]) at N=4M points x in [0,1), dots with
coeffs, and adds linear_weight*x + bias.  On [0,1) this collapses to a
4-piece cubic polynomial, written here in truncated-power form:

    f(x) = a0 + a1*x + a2*x^2 + a3*x^3 + sum_{k=1..3} e_k * relu(x - t_k)^3

with t_k in {0.25, 0.5, 0.75}.  The 7 constants are derived on host in
float64 from (coeffs, linear_weight, bias) and baked into the kernel as
instruction immediates (the NEFF is traced+compiled per call).

Per tile the device computes (1 ACT + 3 custom-DVE passes):
  ACT:  r1 = relu(g1*x - g1*t1)            g1 = |e1|^(1/3)  (so r1^3 = |e1| relu(x-t1)^3)
  DVE1: w  = e2*relu(x-t2)^3 +- r1^3                        (CUBE_PAIR_{P,N}_BS)
  DVE2: v  = e3*relu(x-t3)^3 + w + a0                       (CUBEACC_BS)
  DVE3: y  = (a3*x + a2)*x^2 + a1*x + v                     (POLYADD_BS)

Sharding: x is split into 8 equal contiguous chunks along N, one per core
(embarrassingly data-parallel).
"""

import numpy as np

N_POINTS = 4_194_304
N_CORES = 8
N_PER_CORE = N_POINTS // N_CORES  # 524288
P = 128
F_TOTAL = N_PER_CORE // P  # 4096
# Tile free-dim schedule: small first tile (compute starts sooner) and small
# last tile (final output DMA drains sooner); big middle tiles amortize
# per-instruction overhead.  Must sum to F_TOTAL.
TILES = [384, 1024, 1024, 1024, 640]
STRIP_INIT = False

_OPS_CACHE = None


def _get_custom_ops():
    """Build + register the custom DVE ops (idempotent)."""
    global _OPS_CACHE
    if _OPS_CACHE is not None:
        return _OPS_CACHE

    import concourse.dve_ops as dve_ops_mod
    from concourse.dve_ops import DveOp, OPS
    from concourse.dve_spec import (
        Spec, Src0, Src1, C0, C1, C2, relu, sq, lower, _has_src1,
    )
    from concourse.dve_uop import DveOpSpec

    def make(name, body, reference):
        spec = Spec(body=body, reference=reference)
        shas = {
            ver: DveOpSpec(
                name=name, uops=lower(spec, ver=ver), rd1_en=_has_src1(spec)
            ).sha(ver)
            for ver in ("v3", "v4")
        }
        op = DveOp(name, spec, subdim=False, uops_sha=shas)
        OPS.append(op)
        row = dve_ops_mod._CUSTOM_DVE_ROW_BASE + len(OPS) - 1
        assert row < 0x20, "custom DVE row overflow"
        dve_ops_mod._SUB_OPCODE_FOR_NAME[name] = row
        dve_ops_mod.CUSTOM_DVE_SPECS[name] = spec
        return op

    def _relu3(x, s):
        return np.maximum(x.astype(np.float32) - s, 0) ** 3

    _r = relu(Src0 - C0)
    _t2cube = sq(_r) * _r * C1
    _r1cube = sq(Src1) * Src1
    # w = s1*relu(in0-s0)^3 + in1^3
    cube_pair_p = make(
        "CUBE_PAIR_P_BS",
        _t2cube + _r1cube,
        lambda in0, in1, s0, s1, imm2: (
            _relu3(in0, s0) * s1 + in1.astype(np.float32) ** 3
        ).astype(np.float32),
    )
    # w = s1*relu(in0-s0)^3 - in1^3
    cube_pair_n = make(
        "CUBE_PAIR_N_BS",
        _t2cube - _r1cube,
        lambda in0, in1, s0, s1, imm2: (
            _relu3(in0, s0) * s1 - in1.astype(np.float32) ** 3
        ).astype(np.float32),
    )
    # v = s1*relu(in0 - s0)^3 + in1 + imm2
    _r2 = relu(Src0 - C0)
    cubeacc = make(
        "CUBEACC_BS",
        sq(_r2) * _r2 * C1 + Src1 + C2,
        lambda in0, in1, s0, s1, imm2: (
            _relu3(in0, s0) * s1 + in1 + imm2
        ).astype(np.float32),
    )
    # y = (s0*in0 + s1)*in0^2 + imm2*in0 + in1
    polyadd = make(
        "POLYADD_BS",
        (Src0 * C0 + C1) * sq(Src0) + Src0 * C2 + Src1,
        lambda in0, in1, s0, s1, imm2: (
            (in0.astype(np.float32) * s0 + s1) * in0.astype(np.float32) ** 2
            + in0.astype(np.float32) * imm2
            + in1
        ).astype(np.float32),
    )
    # p = (s0*in0 + s1)*in0^2 + imm2*in0   (no second stream; pure-DVE pipeline fill)
    polyimm = make(
        "POLY_IMM_BS",
        (Src0 * C0 + C1) * sq(Src0) + Src0 * C2,
        lambda in0, in1, s0, s1, imm2: (
            (in0.astype(np.float32) * s0 + s1) * in0.astype(np.float32) ** 2
            + in0.astype(np.float32) * imm2
        ).astype(np.float32),
    )
    _OPS_CACHE = (cube_pair_p, cube_pair_n, cubeacc, polyadd, polyimm)
    return _OPS_CACHE


def _derive_constants(coeffs, linear_weight, bias):
    """float64 host derivation of (a0..a3, e1..e3) from the tiny inputs."""
    from math import comb

    c = np.asarray(coeffs, dtype=np.float64).reshape(-1)
    lw = float(np.asarray(linear_weight, dtype=np.float64).reshape(-1)[0])
    b = float(np.asarray(bias, dtype=np.float64).reshape(-1)[0])

    # Cardinal cubic B-spline weights as polynomials in t (columns: 1,t,t^2,t^3)
    W = np.array(
        [[1, -3, 3, -1], [4, 0, -6, 3], [1, 3, 3, -3], [0, 0, 0, 1]],
        dtype=np.float64,
    ) / 6.0
    pieces = []
    for j in range(4):  # interval [j/4, (j+1)/4)
        pt = np.zeros(4)
        for m in range(4):
            pt += c[4 + j + m] * W[m]
        # substitute t = 4x - j
        px = np.zeros(4)
        for k in range(4):
            for i in range(k + 1):
                px[i] += pt[k] * comb(k, i) * (4.0 ** i) * ((-float(j)) ** (k - i))
        pieces.append(px)
    a = pieces[0].copy()
    a[0] += b
    a[1] += lw
    e = [pieces[j][3] - pieces[j - 1][3] for j in range(1, 4)]
    return (a[0], a[1], a[2], a[3], e[0], e[1], e[2])


def _split_sync_waits(nc, max_waits=1):
    """Workaround: this container's walrus accepts only one sync-wait per
    instruction (setupSyncWait 'Too many sync wait commands').  Hoist extra
    waits onto preceding same-engine NoOps (AND-semantics preserved)."""
    import concourse.mybir as mybir

    cnt = 0
    for bb in nc.main_func.blocks:
        insts = bb.instructions
        new_list = []
        changed = False
        for inst in insts:
            si = getattr(inst, "sync_info", None)
            if si is not None and si.on_wait and len(si.on_wait) > max_waits:
                waits = list(si.on_wait)
                extra, keep = waits[:-max_waits], waits[-max_waits:]
                for j in range(0, len(extra), max_waits):
                    chunk = extra[j : j + max_waits]
                    nop = mybir.InstNoOp(
                        name=f"waitsplit-{cnt}",
                        engine=inst.engine,
                        sync_info=mybir.SyncInfo(on_wait=list(chunk), on_update=[]),
                    )
                    cnt += 1
                    new_list.append(nop)
                si.on_wait = keep
                changed = True
            new_list.append(inst)
        if changed:
            bb.instructions = new_list
    return cnt


def _cheap_drain_and_barrier(self, tick_clock, wait_clock):
    """Cheaper TileContext tail. The stock tail is
    drain + all_engine_barrier + sem_clear + all_engine_barrier (two EVSEM
    butterflies, ~9us wall).  Replacement: SP drains with the global-clock
    waits (covers DMA completion) and incs a gather sem, the other compute
    engines drain + inc, and gpsimd (after its own drain) waits for all 4
    then clears every tile semaphore (+ the gather sem) so the NEFF stays
    re-executable.  No release broadcast is needed: NRT won't start a next
    execution until every engine's stream has ended."""
    from concourse.vector_clock import ScopedClock

    nc = self.nc
    drain_inst = nc.sync.drain()
    wait_clock.add_sem_waits(
        drain_inst.ins, ScopedClock({None: tick_clock.global_clock})
    )
    # Keep only the DMA-queue completion waits: every compute sem is implied
    # transitively by the output DMAs' data dependencies (the HWDGE queue
    # only launches a transfer after its producers signalled), so the DMA
    # completion sems are the unique "ends" of the dependency DAG.
    si = drain_inst.ins.sync_info
    if si is not None and si.on_wait:
        dma_waits = [w for w in si.on_wait if "DMA" in (w.ant_name or "")]
        if dma_waits:
            si.on_wait = dma_waits
    gather = nc.alloc_semaphore("cheap_tail_gather")
    drain_inst.then_inc(gather, 1)
    for eng in (nc.tensor, nc.scalar, nc.vector):
        eng.drain().then_inc(gather, 1)
    nc.gpsimd.drain()
    nc.gpsimd.wait_ge(gather, 4)

    assert self.sems is not None
    popped = nc._tile_sem_poison_stack.pop()
    assert popped is self._sem_poison
    sem_nums = sorted(
        {
            (s.num if hasattr(s, "num") else int(s))
            for s in self.sems.allocated().values()
        }
        | {gather.num}
    )
    from concourse.bass import compact_to_ranges

    for rng in compact_to_ranges(sem_nums):
        nc.gpsimd.dma_reset(rng)
        nc.gpsimd.sem_clear(rng)


def _build_bass(consts, tiles=None, strip_init=None):
    import concourse.bass as bass
    import concourse.mybir as mybir
    from concourse.tile import TileContext

    if tiles is None:
        tiles = TILES
    if strip_init is None:
        strip_init = STRIP_INIT
    assert sum(tiles) == F_TOTAL, (tiles, F_TOTAL)

    a0, a1, a2, a3, e1, e2, e3 = (float(v) for v in consts)
    cube_pair_p, cube_pair_n, cubeacc, polyadd, polyimm = _get_custom_ops()
    cube_pair = cube_pair_p if e1 >= 0 else cube_pair_n
    g1 = abs(e1) ** (1.0 / 3.0)
    bias1 = -0.25 * g1

    TileContext._drain_and_barrier = _cheap_drain_and_barrier

    nc = bass.Bass("TRN2", debug=False)
    # Bass.__init__ emits const-AP memsets (unused here) plus a full
    # all-engine barrier; both sit on the critical path before the first
    # DMA can issue.  Record them for post-trace removal.
    strip_names = frozenset(
        inst.name
        for bb in nc.main_func.blocks
        for inst in bb.instructions
        if type(inst).__name__ in ("InstMemset", "InstDrain", "InstEventSemaphore")
    ) if strip_init else frozenset()

    f32 = mybir.dt.float32
    x_t = nc.dram_tensor("x", [N_PER_CORE, 1], f32, kind="ExternalInput")
    y_t = nc.dram_tensor("y", [N_PER_CORE, 1], f32, kind="ExternalOutput")
    Relu = mybir.ActivationFunctionType.Relu

    # Each tile t is a contiguous x-range of P*F_t elements viewed as [P, F_t].
    xa = x_t.ap()
    ya = y_t.ap()

    def tile_view(ap, start, f):
        # contiguous range [start*P .. start*P + P*f) as [P, f]
        return ap[start : start + P * f].rearrange("(p f) o -> p (f o)", p=P, f=f)

    with TileContext(nc) as tc:
        with tc.tile_pool(name="pool", bufs=6) as pool:
            bias_t = pool.tile([P, 1], f32, tag="bias")
            nc.gpsimd.memset(bias_t[:], bias1)
            off = 0
            for t, ftile in enumerate(tiles):
                xv = tile_view(xa, off, ftile)
                yv = tile_view(ya, off, ftile)
                off += P * ftile
                xt = pool.tile([P, ftile], f32, tag="x")
                nc.sync.dma_start(out=xt[:], in_=xv)
                if t == 0:
                    # Pipeline-fill tile: pure-DVE 4-pass chain with no ACT
                    # dependency, so compute starts as soon as the first DMA
                    # lands (ACT table load + first ACTIVATE are off the
                    # critical path).
                    pp = pool.tile([P, ftile], f32, tag="w")
                    nc.vector._custom_dve(
                        polyimm, out=pp[:], in0=xt[:], s0=a3, s1=a2, imm2=a1
                    )
                    v1 = pool.tile([P, ftile], f32, tag="v")
                    nc.vector._custom_dve(
                        cubeacc, out=v1[:], in0=xt[:], in1=pp[:],
                        s0=0.25, s1=e1, imm2=a0,
                    )
                    v2 = pool.tile([P, ftile], f32, tag="r1")
                    nc.vector._custom_dve(
                        cubeacc, out=v2[:], in0=xt[:], in1=v1[:],
                        s0=0.5, s1=e2, imm2=0.0,
                    )
                    o = pool.tile([P, ftile], f32, tag="o")
                    nc.vector._custom_dve(
                        cubeacc, out=o[:], in0=xt[:], in1=v2[:],
                        s0=0.75, s1=e3, imm2=0.0,
                    )
                else:
                    r1 = pool.tile([P, ftile], f32, tag="r1")
                    nc.scalar.activation(
                        r1[:], xt[:], Relu, bias=bias_t[:], scale=g1
                    )
                    w = pool.tile([P, ftile], f32, tag="w")
                    nc.vector._custom_dve(
                        cube_pair, out=w[:], in0=xt[:], in1=r1[:], s0=0.5, s1=e2
                    )
                    v = pool.tile([P, ftile], f32, tag="v")
                    nc.vector._custom_dve(
                        cubeacc, out=v[:], in0=xt[:], in1=w[:],
                        s0=0.75, s1=e3, imm2=a0,
                    )
                    o = pool.tile([P, ftile], f32, tag="o")
                    nc.vector._custom_dve(
                        polyadd, out=o[:], in0=xt[:], in1=v[:],
                        s0=a3, s1=a2, imm2=a1,
                    )
                nc.sync.dma_start(out=yv, in_=o[:])
    # Populate .instr bytes for InstISA subclasses (InstCustomDveAnt).
    # Raw Bass doesn't run this pass; without it walrus codegen sees an
    # empty .instr and fails with "ISA wrong length".
    mybir.codegen_inst_isa_subclasses(nc)
    _split_sync_waits(nc, max_waits=1)
    if strip_names:
        for bb in nc.main_func.blocks:
            bb.instructions = [
                i for i in bb.instructions if i.name not in strip_names
            ]
    return nc


def run(x, coeffs, linear_weight, bias, trace=False, trace_kwargs=None):
    """Compile + run on 8 cores; returns (output, BassKernelResults)."""
    from concourse.bass_utils import run_bass_kernel_spmd

    consts = _derive_constants(coeffs, linear_weight, bias)
    nc = _build_bass(consts)

    x_np = np.ascontiguousarray(np.asarray(x, dtype=np.float32)).reshape(
        N_CORES, N_PER_CORE, 1
    )
    in_maps = [{"x": x_np[i]} for i in range(N_CORES)]
    kwargs = {}
    if trace:
        kwargs["trace"] = True
        if trace_kwargs:
            kwargs.update(trace_kwargs)
    res = run_bass_kernel_spmd(nc, in_maps, core_ids=list(range(N_CORES)), **kwargs)
    out = np.concatenate([r["y"] for r in res.results], axis=0).astype(np.float32)
    return out, res


def kernel(x, coeffs, linear_weight, bias):
    out, _ = run(x, coeffs, linear_weight, bias, trace=False)
    return out
